# revision 11
# baseline (speedup 1.0000x reference)
"""Trainium2 Bass kernel for BinaryCE + rejection-softmax loss.

Reference computation (B=256, C=500, D=256):
    y = labels.astype(f32)                                   # [B, C]
    bce[b] = sum_c( softplus(logits) - y*logits )            # log-sigmoid BCE
    max_sim[b, c] = max_d wf[c, b, d]
    rej[b] = sum_c (labels==0) * relu(sigmoid(max_sim) - 0.3)
    out[b] = bce[b] + rej[b]

Sharding: data-parallel over B across 8 cores (wf on axis 1,
logits/labels on axis 0). Per core: logits [32,500], wf [500,32,256]
(zero-padded to [512,32,256] on the host while slicing), labels
[32,500] -> out [32]. No cross-device reduction.

Layout: the wf slice is viewed as [128 partitions, 32768] with
partition p holding the 4 consecutive classes c = 4p..4p+3 — each
partition reads one fully contiguous 128 KB run, which is the only
DMA shape this part streams at full HBM rate (non-128-partition or
short-run patterns measured 110-160 GB/s vs ~350 GB/s). The stream is
split into c4-stripes so the DVE max-reduce pipelines behind the DMA.
max_sim lands as [128, 4, 32] (c = 4p + c4); the label mask is built
in the same layout with four stride-4 PE transposes; per-class sums
collapse through a ones-vector matmul into PSUM [1, 32], with the BCE
column injected via an identity-matmul transpose.
"""

import sys

for _p in ("/root/.axon_site", "/root/.axon_site/_ro/trn_rl_repo",
           "/root/.axon_site/_ro/pypackages", "/opt/trn_rl_repo"):
    if _p not in sys.path:
        sys.path.append(_p)

import numpy as np

import concourse.bass as bass  # noqa: F401  (registers engine classes)
import concourse.tile as tile
from concourse import bacc, mybir
from concourse.bass_utils import run_bass_kernel_spmd
from concourse.masks import make_identity

F32 = mybir.dt.float32
I32 = mybir.dt.int32
AF = mybir.ActivationFunctionType
ALU = mybir.AluOpType
AX = mybir.AxisListType

B, C, D = 256, 500, 256
REJECTION_MARGIN = 0.3
NCORES = 8
BL = B // NCORES          # 32 samples per core
NP = 128                  # partitions
C4 = 4                    # classes per partition
CP = NP * C4              # 512 padded classes
NCHUNK = 8                # wf stream chunks ([128, 4096] each, 16 KB runs)
CH = CP * BL * D // NP // NCHUNK   # 4096 elems/partition/chunk

WF_BUFS = 6


def build_nc(debug: bool = False):
    nc = bacc.Bacc("TRN2", target_bir_lowering=False, debug=debug)

    logits_d = nc.dram_tensor("logits", [BL, C], F32, kind="ExternalInput")
    wf_d = nc.dram_tensor("wf", [CP, BL, D], F32, kind="ExternalInput")
    labels_d = nc.dram_tensor("labels", [BL, C], I32, kind="ExternalInput")
    out_d = nc.dram_tensor("out", [1, BL], F32, kind="ExternalOutput")

    # [128, 32768]: partition p = classes 4p..4p+3, contiguous per partition
    wfv = wf_d[:].rearrange("(p c4) b d -> p (c4 b d)", c4=C4)

    with tile.TileContext(nc) as tc:
        with (
            tc.tile_pool(name="consts", bufs=1) as consts,
            tc.tile_pool(name="wfp", bufs=WF_BUFS) as wfp,
            tc.tile_pool(name="chunk", bufs=2) as chunkp,
            tc.tile_pool(name="psum_t", bufs=2, space="PSUM") as psum_t,
            tc.tile_pool(name="psum_acc", bufs=1, space="PSUM") as psum_acc,
        ):
            # --- small inputs first on the SP ring (tiny) -------------------
            logits_sb = consts.tile([BL, C], F32)
            nc.sync.dma_start(logits_sb[:], logits_d[:])
            labels_sb = consts.tile([BL, C], I32)
            nc.sync.dma_start(labels_sb[:], labels_d[:])

            # identity on gpsimd: nothing else queues on the Q7 now.
            ident = consts.tile([BL, BL], F32)
            make_identity(nc, ident[:])

            # --- wf stream over both HWDGE rings (SP + ACT) -----------------
            # HW descriptor generation starts within ~1 us (no Q7 descgen
            # latency); two rings together sustain ~340+ GB/s on this
            # 128-partition contiguous-run pattern. The last two chunks are
            # half-size so the final max-reduce tail is short.
            msim = consts.tile([NP, C4, BL], F32)
            chunks = [(k * CH, CH) for k in range(NCHUNK - 1)]
            chunks += [((NCHUNK - 1) * CH, CH // 2),
                       ((NCHUNK - 1) * CH + CH // 2, CH // 2)]
            for k, (off, ln) in enumerate(chunks):
                wft = wfp.tile([NP, CH], F32, tag="wft")
                eng = nc.sync if k % 2 == 0 else nc.scalar
                eng.dma_start(wft[:, :ln], wfv[:, off:off + ln])
                c4 = off // (BL * D)
                boff = off % (BL * D) // D
                nc.vector.reduce_max(
                    msim[:, c4, boff:boff + ln // D],
                    wft[:, :ln].rearrange("p (b d) -> p b d", d=D), axis=AX.X)

            labels_f = consts.tile([BL, C], F32)
            nc.vector.tensor_copy(labels_f[:], labels_sb[:])

            ones = consts.tile([NP, 1], F32)
            nc.vector.memset(ones[:], 1.0)
            neg_margin = consts.tile([NP, 1], F32)
            nc.vector.memset(neg_margin[:], -REJECTION_MARGIN)

            # --- BCE part in natural [b, c] layout -------------------------
            # softplus(x) = ln(exp(x) + 1); no Softplus LUT on TRN2.
            # Safe: |logits| <~ 5 so exp() cannot overflow.
            exp_tmp = consts.tile([BL, C], F32)
            nc.scalar.activation(exp_tmp[:], logits_sb[:], AF.Exp)
            sp_tmp = consts.tile([BL, C], F32)
            sp_sum = consts.tile([BL, 1], F32)
            nc.scalar.activation(sp_tmp[:], exp_tmp[:], AF.Ln, bias=1.0,
                                 accum_out=sp_sum[:])
            yx_tmp = consts.tile([BL, C], F32)
            yx_sum = consts.tile([BL, 1], F32)
            nc.vector.tensor_mul(yx_tmp[:], labels_f[:], logits_sb[:])
            nc.vector.reduce_sum(yx_sum[:], yx_tmp[:], axis=AX.X)
            bce_col = consts.tile([BL, 1], F32)
            nc.vector.tensor_sub(bce_col[:], sp_sum[:], yx_sum[:])

            # --- mask = 1 - labels^T in [p, c4, b] layout (c = 4p + c4) ----
            # Padded classes c >= 500 keep mask 0 from the memset, so the
            # zero-padded wf rows contribute nothing.
            mask_sb = consts.tile([NP, C4, BL], F32)
            nc.vector.memset(mask_sb[:], 0.0)
            for c4 in range(C4):
                labT = psum_t.tile([C // C4, BL], F32, tag="labT")
                nc.tensor.matmul(labT[:], labels_f[:, c4::C4], ident[:],
                                 start=True, stop=True)
                nc.scalar.activation(mask_sb[:C // C4, c4, :], labT[:],
                                     AF.Identity, bias=1.0, scale=-1.0)

            # --- PSUM accumulator [1, 32]; BCE row first -------------------
            acc = psum_acc.tile([1, BL], F32)
            nc.tensor.matmul(acc[:], bce_col[:], ident[:],
                             start=True, stop=False)

            # --- masked rejection, summed over classes via ones-matmul -----
            for c4 in range(C4):
                sig = chunkp.tile([NP, BL], F32, tag="sig")
                nc.scalar.activation(sig[:], msim[:, c4, :], AF.Sigmoid)
                rej = chunkp.tile([NP, BL], F32, tag="rej")
                nc.scalar.activation(rej[:], sig[:], AF.Relu,
                                     bias=neg_margin[:])
                rejm = chunkp.tile([NP, BL], F32, tag="rejm")
                nc.vector.tensor_mul(rejm[:], rej[:], mask_sb[:, c4, :])
                nc.tensor.matmul(acc[:], ones[:], rejm[:],
                                 start=False, stop=(c4 == C4 - 1))

            out_sb = consts.tile([1, BL], F32)
            nc.scalar.copy(out_sb[:], acc[:])
            nc.sync.dma_start(out_d[:], out_sb[:])

    nc.compile()
    return nc


_NC_CACHE = None


def _get_nc():
    global _NC_CACHE
    if _NC_CACHE is None:
        _NC_CACHE = build_nc()
    return _NC_CACHE


def _in_maps(logits, wf, labels):
    maps = []
    for k in range(NCORES):
        b0 = k * BL
        wf_pad = np.zeros((CP, BL, D), dtype=np.float32)
        wf_pad[:C] = wf[:, b0:b0 + BL, :]
        maps.append({
            "logits": np.ascontiguousarray(logits[b0:b0 + BL]),
            "wf": wf_pad,
            "labels": np.ascontiguousarray(labels[b0:b0 + BL]),
        })
    return maps


def run(logits, wf, labels, trace: bool = False, tmpdir: str | None = None):
    """Run on all 8 cores; returns (full_output [B], BassKernelResults)."""
    logits = np.asarray(logits, dtype=np.float32)
    wf = np.asarray(wf, dtype=np.float32)
    labels = np.asarray(labels, dtype=np.int32)
    assert logits.shape == (B, C) and wf.shape == (C, B, D) \
        and labels.shape == (B, C)

    nc = _get_nc()
    res = run_bass_kernel_spmd(nc, _in_maps(logits, wf, labels),
                               list(range(NCORES)), trace=trace,
                               tmpdir=tmpdir)
    out = np.concatenate(
        [np.asarray(res.results[k]["out"]).reshape(BL) for k in range(NCORES)])
    return out.astype(np.float32), res


def kernel(logits, wf, labels):
    out, _ = run(logits, wf, labels)
    return out


# revision 12
# speedup vs baseline: 1.0558x; 1.0558x over previous
"""Trainium2 Bass kernel for BinaryCE + rejection-softmax loss.

Reference computation (B=256, C=500, D=256):
    y = labels.astype(f32)                                   # [B, C]
    bce[b] = sum_c( softplus(logits) - y*logits )            # log-sigmoid BCE
    max_sim[b, c] = max_d wf[c, b, d]
    rej[b] = sum_c (labels==0) * relu(sigmoid(max_sim) - 0.3)
    out[b] = bce[b] + rej[b]

Sharding: data-parallel over B across 8 cores (wf on axis 1,
logits/labels on axis 0). Per core: logits [32,500], wf [500,32,256]
(zero-padded to [512,32,256] on the host while slicing), labels
[32,500] -> out [32]. No cross-device reduction.

Layout: the wf slice is viewed as [128 partitions, 32768] with
partition p holding the 4 consecutive classes c = 4p..4p+3 — each
partition reads one fully contiguous 128 KB run, which is the only
DMA shape this part streams at full HBM rate (non-128-partition or
short-run patterns measured 110-160 GB/s vs ~350 GB/s). The stream is
split into c4-stripes so the DVE max-reduce pipelines behind the DMA.
max_sim lands as [128, 4, 32] (c = 4p + c4); the label mask is built
in the same layout with four stride-4 PE transposes; per-class sums
collapse through a ones-vector matmul into PSUM [1, 32], with the BCE
column injected via an identity-matmul transpose.
"""

import sys

for _p in ("/root/.axon_site", "/root/.axon_site/_ro/trn_rl_repo",
           "/root/.axon_site/_ro/pypackages", "/opt/trn_rl_repo"):
    if _p not in sys.path:
        sys.path.append(_p)

import numpy as np

import concourse.bass as bass  # noqa: F401  (registers engine classes)
import concourse.tile as tile
from concourse import bacc, mybir
from concourse.bass_utils import run_bass_kernel_spmd
from concourse.masks import make_identity

F32 = mybir.dt.float32
I32 = mybir.dt.int32
AF = mybir.ActivationFunctionType
ALU = mybir.AluOpType
AX = mybir.AxisListType

B, C, D = 256, 500, 256
REJECTION_MARGIN = 0.3
NCORES = 8
BL = B // NCORES          # 32 samples per core
NP = 128                  # partitions
C4 = 4                    # classes per partition
CP = NP * C4              # 512 padded classes
NCHUNK = 8                # wf stream chunks ([128, 4096] each, 16 KB runs)
CH = CP * BL * D // NP // NCHUNK   # 4096 elems/partition/chunk

WF_BUFS = 6


def build_nc(debug: bool = False):
    nc = bacc.Bacc("TRN2", target_bir_lowering=False, debug=debug)

    logits_d = nc.dram_tensor("logits", [BL, C], F32, kind="ExternalInput")
    wf_d = nc.dram_tensor("wf", [CP, BL, D], F32, kind="ExternalInput")
    labels_d = nc.dram_tensor("labels", [BL, C], I32, kind="ExternalInput")
    out_d = nc.dram_tensor("out", [1, BL], F32, kind="ExternalOutput")

    # [128, 32768]: partition p = classes 4p..4p+3, contiguous per partition
    wfv = wf_d[:].rearrange("(p c4) b d -> p (c4 b d)", c4=C4)

    with tile.TileContext(nc) as tc:
        with (
            tc.tile_pool(name="consts", bufs=1) as consts,
            tc.tile_pool(name="wfp", bufs=WF_BUFS) as wfp,
            tc.tile_pool(name="chunk", bufs=2) as chunkp,
            tc.tile_pool(name="psum_t", bufs=2, space="PSUM") as psum_t,
            tc.tile_pool(name="psum_acc", bufs=1, space="PSUM") as psum_acc,
        ):
            # --- small inputs first on the SP ring (tiny) -------------------
            logits_sb = consts.tile([BL, C], F32)
            nc.sync.dma_start(logits_sb[:], logits_d[:])
            labels_sb = consts.tile([BL, C], I32)
            nc.sync.dma_start(labels_sb[:], labels_d[:])

            # --- wf stream ---------------------------------------------------
            # Bulk goes through the single SWDGE queue (gpsimd-issued): the
            # only path that sustains ~400 GB/s here. Each HWDGE ring caps at
            # ~160-190 GB/s, so the HW rings (SP/ACT) only carry the first
            # two half-chunks — they start ~2 us before the Q7 descgen warms
            # up. The last two chunks are half-size to shorten the final
            # max-reduce tail.
            msim = consts.tile([NP, C4, BL], F32)
            H = CH // 2
            chunks = [(0, H, nc.sync), (H, H, nc.scalar)]
            chunks += [(k * CH, CH, nc.gpsimd) for k in range(1, NCHUNK - 1)]
            chunks += [((NCHUNK - 1) * CH, H, nc.gpsimd),
                       ((NCHUNK - 1) * CH + H, H, nc.gpsimd)]
            for off, ln, eng in chunks:
                wft = wfp.tile([NP, CH], F32, tag="wft")
                eng.dma_start(wft[:, :ln], wfv[:, off:off + ln])
                c4 = off // (BL * D)
                boff = off % (BL * D) // D
                nc.vector.reduce_max(
                    msim[:, c4, boff:boff + ln // D],
                    wft[:, :ln].rearrange("p (b d) -> p b d", d=D), axis=AX.X)

            # identity after the descgens: gpsimd program order would
            # otherwise delay the first wf chunk by the Q7 launches.
            ident = consts.tile([BL, BL], F32)
            make_identity(nc, ident[:])

            labels_f = consts.tile([BL, C], F32)
            nc.vector.tensor_copy(labels_f[:], labels_sb[:])

            ones = consts.tile([NP, 1], F32)
            nc.vector.memset(ones[:], 1.0)
            neg_margin = consts.tile([NP, 1], F32)
            nc.vector.memset(neg_margin[:], -REJECTION_MARGIN)

            # --- BCE part in natural [b, c] layout -------------------------
            # softplus(x) = ln(exp(x) + 1); no Softplus LUT on TRN2.
            # Safe: |logits| <~ 5 so exp() cannot overflow.
            exp_tmp = consts.tile([BL, C], F32)
            nc.scalar.activation(exp_tmp[:], logits_sb[:], AF.Exp)
            sp_tmp = consts.tile([BL, C], F32)
            sp_sum = consts.tile([BL, 1], F32)
            nc.scalar.activation(sp_tmp[:], exp_tmp[:], AF.Ln, bias=1.0,
                                 accum_out=sp_sum[:])
            yx_tmp = consts.tile([BL, C], F32)
            yx_sum = consts.tile([BL, 1], F32)
            nc.vector.tensor_mul(yx_tmp[:], labels_f[:], logits_sb[:])
            nc.vector.reduce_sum(yx_sum[:], yx_tmp[:], axis=AX.X)
            bce_col = consts.tile([BL, 1], F32)
            nc.vector.tensor_sub(bce_col[:], sp_sum[:], yx_sum[:])

            # --- mask = 1 - labels^T in [p, c4, b] layout (c = 4p + c4) ----
            # Padded classes c >= 500 keep mask 0 from the memset, so the
            # zero-padded wf rows contribute nothing.
            mask_sb = consts.tile([NP, C4, BL], F32)
            nc.vector.memset(mask_sb[:], 0.0)
            for c4 in range(C4):
                labT = psum_t.tile([C // C4, BL], F32, tag="labT")
                nc.tensor.matmul(labT[:], labels_f[:, c4::C4], ident[:],
                                 start=True, stop=True)
                nc.scalar.activation(mask_sb[:C // C4, c4, :], labT[:],
                                     AF.Identity, bias=1.0, scale=-1.0)

            # --- PSUM accumulator [1, 32]; BCE row first -------------------
            acc = psum_acc.tile([1, BL], F32)
            nc.tensor.matmul(acc[:], bce_col[:], ident[:],
                             start=True, stop=False)

            # --- masked rejection, summed over classes via ones-matmul -----
            for c4 in range(C4):
                sig = chunkp.tile([NP, BL], F32, tag="sig")
                nc.scalar.activation(sig[:], msim[:, c4, :], AF.Sigmoid)
                rej = chunkp.tile([NP, BL], F32, tag="rej")
                nc.scalar.activation(rej[:], sig[:], AF.Relu,
                                     bias=neg_margin[:])
                rejm = chunkp.tile([NP, BL], F32, tag="rejm")
                nc.vector.tensor_mul(rejm[:], rej[:], mask_sb[:, c4, :])
                nc.tensor.matmul(acc[:], ones[:], rejm[:],
                                 start=False, stop=(c4 == C4 - 1))

            out_sb = consts.tile([1, BL], F32)
            nc.scalar.copy(out_sb[:], acc[:])
            nc.sync.dma_start(out_d[:], out_sb[:])

    nc.compile()
    return nc


_NC_CACHE = None


def _get_nc():
    global _NC_CACHE
    if _NC_CACHE is None:
        _NC_CACHE = build_nc()
    return _NC_CACHE


def _in_maps(logits, wf, labels):
    maps = []
    for k in range(NCORES):
        b0 = k * BL
        wf_pad = np.zeros((CP, BL, D), dtype=np.float32)
        wf_pad[:C] = wf[:, b0:b0 + BL, :]
        maps.append({
            "logits": np.ascontiguousarray(logits[b0:b0 + BL]),
            "wf": wf_pad,
            "labels": np.ascontiguousarray(labels[b0:b0 + BL]),
        })
    return maps


def run(logits, wf, labels, trace: bool = False, tmpdir: str | None = None):
    """Run on all 8 cores; returns (full_output [B], BassKernelResults)."""
    logits = np.asarray(logits, dtype=np.float32)
    wf = np.asarray(wf, dtype=np.float32)
    labels = np.asarray(labels, dtype=np.int32)
    assert logits.shape == (B, C) and wf.shape == (C, B, D) \
        and labels.shape == (B, C)

    nc = _get_nc()
    res = run_bass_kernel_spmd(nc, _in_maps(logits, wf, labels),
                               list(range(NCORES)), trace=trace,
                               tmpdir=tmpdir)
    out = np.concatenate(
        [np.asarray(res.results[k]["out"]).reshape(BL) for k in range(NCORES)])
    return out.astype(np.float32), res


def kernel(logits, wf, labels):
    out, _ = run(logits, wf, labels)
    return out


# revision 15
# speedup vs baseline: 1.1879x; 1.1251x over previous
"""Trainium2 Bass kernel for BinaryCE + rejection-softmax loss.

Reference computation (B=256, C=500, D=256):
    y = labels.astype(f32)                                   # [B, C]
    bce[b] = sum_c( softplus(logits) - y*logits )            # log-sigmoid BCE
    max_sim[b, c] = max_d wf[c, b, d]
    rej[b] = sum_c (labels==0) * relu(sigmoid(max_sim) - 0.3)
    out[b] = bce[b] + rej[b]

Sharding: data-parallel over B across 8 cores (wf on axis 1,
logits/labels on axis 0). Per core: logits [32,500], wf [500,32,256]
(zero-padded to [512,32,256] on the host while slicing), labels
[32,500] -> out [32]. No cross-device reduction.

Layout: the wf slice is viewed as [128 partitions, 32768] with
partition p holding the 4 consecutive classes c = 4p..4p+3 — each
partition reads one fully contiguous 128 KB run, which is the only
DMA shape this part streams at full HBM rate (non-128-partition or
short-run patterns measured 110-160 GB/s vs ~350 GB/s). The stream is
split into c4-stripes so the DVE max-reduce pipelines behind the DMA.
max_sim lands as [128, 4, 32] (c = 4p + c4); the label mask is built
in the same layout with four stride-4 PE transposes; per-class sums
collapse through a ones-vector matmul into PSUM [1, 32], with the BCE
column injected via an identity-matmul transpose.
"""

import sys

for _p in ("/root/.axon_site", "/root/.axon_site/_ro/trn_rl_repo",
           "/root/.axon_site/_ro/pypackages", "/opt/trn_rl_repo"):
    if _p not in sys.path:
        sys.path.append(_p)

import numpy as np

import concourse.bass as bass  # noqa: F401  (registers engine classes)
import concourse.tile as tile
from concourse import bacc, mybir
from concourse.bass_utils import run_bass_kernel_spmd
from concourse.masks import make_identity

F32 = mybir.dt.float32
I32 = mybir.dt.int32
AF = mybir.ActivationFunctionType
ALU = mybir.AluOpType
AX = mybir.AxisListType

B, C, D = 256, 500, 256
REJECTION_MARGIN = 0.3
NCORES = 8
BL = B // NCORES          # 32 samples per core
NP = 128                  # partitions
C4 = 4                    # classes per partition
CP = NP * C4              # 512 padded classes
NCHUNK = 8                # wf stream chunks ([128, 4096] each, 16 KB runs)
CH = CP * BL * D // NP // NCHUNK   # 4096 elems/partition/chunk

WF_BUFS = 6


def build_nc(debug: bool = False):
    nc = bacc.Bacc("TRN2", target_bir_lowering=False, debug=debug)

    logits_d = nc.dram_tensor("logits", [BL, C], F32, kind="ExternalInput")
    wf_d = nc.dram_tensor("wf", [CP, BL, D], F32, kind="ExternalInput")
    labels_d = nc.dram_tensor("labels", [BL, C], I32, kind="ExternalInput")
    out_d = nc.dram_tensor("out", [1, BL], F32, kind="ExternalOutput")

    # [128, 32768]: partition p = classes 4p..4p+3, contiguous per partition
    wfv = wf_d[:].rearrange("(p c4) b d -> p (c4 b d)", c4=C4)

    with tile.TileContext(nc) as tc:
        with (
            tc.tile_pool(name="consts", bufs=1) as consts,
            tc.tile_pool(name="wfp", bufs=WF_BUFS) as wfp,
            tc.tile_pool(name="chunk", bufs=2) as chunkp,
            tc.tile_pool(name="psum_t", bufs=2, space="PSUM") as psum_t,
            tc.tile_pool(name="psum_acc", bufs=1, space="PSUM") as psum_acc,
        ):
            # --- small inputs on the ACT ring (tiny, independent) -----------
            logits_sb = consts.tile([BL, C], F32)
            nc.scalar.dma_start(logits_sb[:], logits_d[:])
            labels_sb = consts.tile([BL, C], I32)
            nc.scalar.dma_start(labels_sb[:], labels_d[:])

            # --- wf stream ---------------------------------------------------
            # Everything through the single SWDGE queue (gpsimd-issued): the
            # only path that sustains ~400 GB/s here (each HWDGE ring caps at
            # ~160-190 GB/s and starves when SWDGE saturates the SDMA
            # engines). The last two chunks are half-size to shorten the
            # final max-reduce tail.
            msim = consts.tile([NP, C4, BL], F32)
            H = CH // 2
            chunks = [(k * CH, CH) for k in range(NCHUNK - 1)]
            chunks += [((NCHUNK - 1) * CH, H),
                       ((NCHUNK - 1) * CH + H, H)]
            for off, ln in chunks:
                wft = wfp.tile([NP, CH], F32, tag="wft")
                nc.gpsimd.dma_start(wft[:, :ln], wfv[:, off:off + ln])
                c4 = off // (BL * D)
                boff = off % (BL * D) // D
                nc.vector.reduce_max(
                    msim[:, c4, boff:boff + ln // D],
                    wft[:, :ln].rearrange("p (b d) -> p b d", d=D), axis=AX.X)

            # identity after the descgens: gpsimd program order would
            # otherwise delay the first wf chunk by the Q7 launches.
            ident = consts.tile([BL, BL], F32)
            make_identity(nc, ident[:])

            labels_f = consts.tile([BL, C], F32)
            nc.vector.tensor_copy(labels_f[:], labels_sb[:])

            ones = consts.tile([NP, 1], F32)
            nc.vector.memset(ones[:], 1.0)
            neg_margin = consts.tile([NP, 1], F32)
            nc.vector.memset(neg_margin[:], -REJECTION_MARGIN)

            # --- BCE part in natural [b, c] layout -------------------------
            # softplus(x) = ln(exp(x) + 1); no Softplus LUT on TRN2.
            # Safe: |logits| <~ 5 so exp() cannot overflow.
            exp_tmp = consts.tile([BL, C], F32)
            nc.scalar.activation(exp_tmp[:], logits_sb[:], AF.Exp)
            sp_tmp = consts.tile([BL, C], F32)
            sp_sum = consts.tile([BL, 1], F32)
            nc.scalar.activation(sp_tmp[:], exp_tmp[:], AF.Ln, bias=1.0,
                                 accum_out=sp_sum[:])
            yx_tmp = consts.tile([BL, C], F32)
            yx_sum = consts.tile([BL, 1], F32)
            nc.vector.tensor_mul(yx_tmp[:], labels_f[:], logits_sb[:])
            nc.vector.reduce_sum(yx_sum[:], yx_tmp[:], axis=AX.X)
            bce_col = consts.tile([BL, 1], F32)
            nc.vector.tensor_sub(bce_col[:], sp_sum[:], yx_sum[:])

            # --- mask = 1 - labels^T in [p, c4, b] layout (c = 4p + c4) ----
            # Padded classes c >= 500 keep mask 0 from the memset, so the
            # zero-padded wf rows contribute nothing.
            mask_sb = consts.tile([NP, C4, BL], F32)
            nc.vector.memset(mask_sb[:], 0.0)
            for c4 in range(C4):
                labT = psum_t.tile([C // C4, BL], F32, tag="labT")
                nc.tensor.matmul(labT[:], labels_f[:, c4::C4], ident[:],
                                 start=True, stop=True)
                nc.scalar.activation(mask_sb[:C // C4, c4, :], labT[:],
                                     AF.Identity, bias=1.0, scale=-1.0)

            # --- PSUM accumulator [1, 32]; BCE row first -------------------
            acc = psum_acc.tile([1, BL], F32)
            nc.tensor.matmul(acc[:], bce_col[:], ident[:],
                             start=True, stop=False)

            # --- masked rejection, summed over classes via ones-matmul -----
            for c4 in range(C4):
                sig = chunkp.tile([NP, BL], F32, tag="sig")
                nc.scalar.activation(sig[:], msim[:, c4, :], AF.Sigmoid)
                rej = chunkp.tile([NP, BL], F32, tag="rej")
                nc.scalar.activation(rej[:], sig[:], AF.Relu,
                                     bias=neg_margin[:])
                rejm = chunkp.tile([NP, BL], F32, tag="rejm")
                nc.vector.tensor_mul(rejm[:], rej[:], mask_sb[:, c4, :])
                nc.tensor.matmul(acc[:], ones[:], rejm[:],
                                 start=False, stop=(c4 == C4 - 1))

            out_sb = consts.tile([1, BL], F32)
            nc.scalar.copy(out_sb[:], acc[:])
            nc.scalar.dma_start(out_d[:], out_sb[:])

    nc.compile()
    return nc


_NC_CACHE = None


def _get_nc():
    global _NC_CACHE
    if _NC_CACHE is None:
        _NC_CACHE = build_nc()
    return _NC_CACHE


def _in_maps(logits, wf, labels):
    maps = []
    for k in range(NCORES):
        b0 = k * BL
        wf_pad = np.zeros((CP, BL, D), dtype=np.float32)
        wf_pad[:C] = wf[:, b0:b0 + BL, :]
        maps.append({
            "logits": np.ascontiguousarray(logits[b0:b0 + BL]),
            "wf": wf_pad,
            "labels": np.ascontiguousarray(labels[b0:b0 + BL]),
        })
    return maps


def run(logits, wf, labels, trace: bool = False, tmpdir: str | None = None):
    """Run on all 8 cores; returns (full_output [B], BassKernelResults)."""
    logits = np.asarray(logits, dtype=np.float32)
    wf = np.asarray(wf, dtype=np.float32)
    labels = np.asarray(labels, dtype=np.int32)
    assert logits.shape == (B, C) and wf.shape == (C, B, D) \
        and labels.shape == (B, C)

    nc = _get_nc()
    res = run_bass_kernel_spmd(nc, _in_maps(logits, wf, labels),
                               list(range(NCORES)), trace=trace,
                               tmpdir=tmpdir)
    out = np.concatenate(
        [np.asarray(res.results[k]["out"]).reshape(BL) for k in range(NCORES)])
    return out.astype(np.float32), res


def kernel(logits, wf, labels):
    out, _ = run(logits, wf, labels)
    return out
